# revision 62
# baseline (speedup 1.0000x reference)
"""Single-head causal attention (B=8, T=2048, C=1024, H=128) on 8 TRN2 cores.

Data-parallel over batch: core b computes attention for x[b].

Device-execution-optimized design:

- The host ships x ALREADY TRANSPOSED as bf16 (x^T [C, T]) so the PE
  consumes it directly: no on-device casts, no per-token dequant
  scales, and no PE transposes of x.  (fp8 DoubleRow was measured on
  hardware at 1.0 cycles/output-column -- 2x flops via the doubled
  contraction, not the 4x the cost model claims -- which makes every
  accuracy-viable fp8 scheme here a wash or a loss vs bf16.)
- All PE matmul operands are bf16 (1 col/cycle at ANY moving width,
  unlike f32r which needs >=256-wide outputs), so the P@V moving width
  is 129 (= H + the row-sum ones column).
- Q^T, K^T, V^T [H, T] = W.T @ x^T as PE matmuls contracting over C
  (8 chunks of 128), bias added during the PSUM->SBUF copyback (single
  DVE tensor_scalar, output bf16).  V^T is PE-transposed back to
  natural V [T, H]; its extra 1.0 column makes the out-matmul carry the
  softmax row-sum for free.
- Scores computed transposed per 512-wide query superblock:
  S^T[k, q] = K^T.T @ Q^T so the softmax reduction lands on the PSUM
  partition dim; P = exp(S^T * scale) on the ACT engine (bf16 out) with
  triangular masking on DVE; one accumulated matmul chain per 128-row
  q-block gives out[q, :H] and the row sum; normalization is a
  per-partition reciprocal multiply on the copyback.  Softmax skips the
  max-subtract: |scores * scale| <= ~0.9 so exp cannot overflow.
- Issue-order pipelining: attention for superblock qs is interleaved
  right after projection t-chunk qs so the big exp batches on the ACT
  engine overlap later PE work.  Out-matmul chains are deferred (sb0 by
  one t-chunk, sb1/sb2 to the end) and the final t-chunk reorders the
  V projection and those deferred chains AFTER the last score matmuls,
  so the PE has work while the ACT engine grinds the last exp batch.
- DMAs are issued in need-order in pieces so the first projection
  matmul gates on ~450KB, not megabytes.
"""

import numpy as np

import concourse.mybir as mybir
import concourse.tile as tile
from concourse import bacc
from concourse.bass_utils import run_bass_kernel_spmd

B, T, C, H = 8, 2048, 1024, 128
P = 128
NCB = C // P  # 8 contraction chunks for the projections
NTB = T // P  # 16 token blocks
TCH = 512  # projection t-chunk width (one PSUM bank)
NTCH = T // TCH  # 4
QSB = 512  # query superblock width for attention
VF = H + 1  # free width of the [V | 1.0] tile
F32 = mybir.dt.float32
BF16 = mybir.dt.bfloat16
F16 = mybir.dt.float16
I8 = mybir.dt.int8
SCALE = float(C) ** -0.5

W_BYTES = P * NCB * 3 * H * 2  # bf16 weights, (p, cb, i, h) layout
CF_BYTES = P * 4 * 4  # f32: bq | bk | bv | 1.0 seed
CB_BYTES = W_BYTES + CF_BYTES
X_BYTES = C * T * 2  # bf16 x^T

N_CORES = 8


def build_program(reps=1):
    nc = bacc.Bacc(
        "TRN2",
        target_bir_lowering=False,
        debug=False,
        enable_asserts=False,
        num_devices=N_CORES,
    )

    xb_d = nc.dram_tensor(
        "xb", (CB_BYTES + X_BYTES,), I8, kind="ExternalInput"
    ).ap()
    # weights: [p, cb, 3, H] bf16; slice (cb, i) = W_i[cb*128 + p, :]
    w_d = xb_d[0:W_BYTES].bitcast(BF16).rearrange(
        "(p cb i h) -> p cb i h", p=P, cb=NCB, i=3
    )
    cf_d = (
        xb_d[W_BYTES:CB_BYTES].bitcast(F32).rearrange("(p c) -> p c", c=4)
    )
    # x^T [C, T] -> [p, cb, t]
    x_d = xb_d[CB_BYTES:].bitcast(BF16).rearrange(
        "(cb p t) -> p cb t", p=P, t=T
    )
    y_d = nc.dram_tensor("y", (T, H), F16, kind="ExternalOutput").ap()

    with tile.TileContext(nc) as tc:
        with (
            tc.tile_pool(name="consts", bufs=1) as consts,
            tc.tile_pool(name="big", bufs=1) as big_pool,
            # P tiles are packed 4 per pool buffer: the tile framework's
            # prologue/epilogue semaphore ladders cost ~115ns per pool
            # buffer on every engine, so fewer, bigger buffers directly
            # shrink the fixed overhead.
            tc.tile_pool(name="ptile", bufs=9) as p_pool,
            tc.tile_pool(name="outs", bufs=4) as out_pool,
            tc.tile_pool(name="psA", bufs=5, space="PSUM") as psA,
            tc.tile_pool(name="psB", bufs=3, space="PSUM") as psB,
        ):
            # DMAs in need-order.  vector finishes its fixed preamble
            # earliest, so it issues the two most critical loads (biases,
            # first weight piece); sync takes the rest of the weights and
            # gpsimd streams x^T (t-chunk 0 in cb-pair pieces, then
            # t-chunks 1-3 whole).
            # PE warm-up: the Tensor engine only reaches its top p-state
            # after ~3us of continuous work, and the first projections
            # otherwise run ~1.5x slow while also waiting on the x DMAs.
            # Burn the DMA wait ramping the clock with throwaway matmuls
            # over a zeroed scratch tile (gated on nothing but a tiny
            # memset that dispatches at ~6us, well before any DMA lands).
            wup = consts.tile([P, TCH], BF16, tag="wup")
            nc.vector.memset(wup, 0.0)
            for _ in range(12):
                psw = psA.tile([P, TCH], F32, tag="A")
                nc.tensor.matmul(
                    psw, wup[:, 0:P], wup, start=True, stop=True
                )

            # All bulk input goes through gpsimd's queue group (the only
            # one that sustains ~270GB/s; sync/scalar groups crawl at
            # ~40GB/s), interleaved in exact consumption order.
            cft = consts.tile([P, 4], F32, tag="cft")
            nc.scalar.dma_start(cft, cf_d)
            wt = consts.tile([P, NCB, 3, H], BF16, tag="wt")
            xt = big_pool.tile([P, NCB, T], BF16, tag="xt")
            wsplit = ((0, 2), (2, 5), (5, 8))
            for cp in range(4):
                if cp < len(wsplit):
                    a, b = wsplit[cp]
                    nc.gpsimd.dma_start(wt[:, a:b], w_d[:, a:b])
                nc.gpsimd.dma_start(
                    xt[:, 2 * cp : 2 * cp + 2, 0:TCH],
                    x_d[:, 2 * cp : 2 * cp + 2, 0:TCH],
                )

            # t-chunks 1 and 2 in cb-pair pieces, interleaved 2:1 --
            # t-chunk 1's consumers have ring-pacing slack, while t-chunk
            # 2 gates qT2 and with it the ACT engine's sb2 exp batch.
            def xpiece(tch, cp):
                nc.gpsimd.dma_start(
                    xt[:, 2 * cp : 2 * cp + 2, tch * TCH : (tch + 1) * TCH],
                    x_d[:, 2 * cp : 2 * cp + 2, tch * TCH : (tch + 1) * TCH],
                )

            for tch, cp in (
                (1, 0), (1, 1), (2, 0), (1, 2), (1, 3), (2, 1), (2, 2), (2, 3),
            ):
                xpiece(tch, cp)
            # mask generated on device from the 1.0 seed column (not
            # needed before ~15us, so issued behind the urgent DMAs)
            utri = consts.tile([P, P], BF16, tag="utri")
            nc.gpsimd.affine_select(
                utri,
                cft[:, 3:4].to_broadcast((P, P)),
                [[1, P]],
                mybir.AluOpType.is_ge,
                0.0,
                base=0,
                channel_multiplier=-1,
            )
            for cp in range(4):  # t-chunk 3 in pieces: the hoisted Q3
                xpiece(3, cp)    # projection pipelines through arrivals

            for _ in range(reps):
                qT = big_pool.tile([P, T], BF16, tag="qT")
                kT = big_pool.tile([P, T], BF16, tag="kT")
                v2 = big_pool.tile([P, NTB, VF], BF16, tag="v2")
                nc.vector.memset(v2[:, :, H : H + 1], 1.0)

                def proj_mm(wi, tch, dst, split=False):
                    tsl = slice(tch * TCH, (tch + 1) * TCH)
                    ps = psA.tile([P, TCH], F32, tag="A")
                    for cb in range(NCB):
                        nc.tensor.matmul(
                            ps,
                            wt[:, cb, wi],
                            xt[:, cb, tsl],
                            start=(cb == 0),
                            stop=(cb == NCB - 1),
                        )
                    nc.vector.tensor_scalar_add(
                        dst[:, tsl], ps, cft[:, wi : wi + 1]
                    )

                def proj_v(tch):
                    # V computed NATURAL directly (lhsT = x block, rhs =
                    # Wv): same matmul columns as the V^T projection but
                    # no PE transposes and no wide copybacks.  The V bias
                    # is NOT applied on device: attention(V + b) ==
                    # attention(V) + b, so the host adds it to the final
                    # output in exact f32.
                    for tb in range(tch * (TCH // P), (tch + 1) * (TCH // P)):
                        po = psB.tile([P, VF], F32, tag="B")
                        for cb in range(NCB):
                            nc.tensor.matmul(
                                po[:, :H],
                                xt[:, cb, tb * P : (tb + 1) * P],
                                wt[:, cb, 2],
                                start=(cb == 0),
                                stop=(cb == NCB - 1),
                            )
                        nc.vector.tensor_copy(v2[:, tb, :H], po[:, :H])

                def scores_group(qs, g, p_tiles):
                    # emit score matmuls + exps for kb in [4g, 4g+4) of
                    # superblock qs; groups of 4 share one P-tile pack
                    pack = p_pool.tile([P, 4, QSB], BF16, tag="P")
                    for kb in range(4 * g, 4 * g + 4):
                        j0 = kb - qs * (QSB // P)  # first valid 128-col block
                        off = max(j0, 0) * P
                        ps = psA.tile([P, QSB], F32, tag="A")
                        nc.tensor.matmul(
                            ps[:, off:],
                            kT[:, kb * P : (kb + 1) * P],
                            qT[:, qs * QSB + off : (qs + 1) * QSB],
                            start=True,
                            stop=True,
                        )
                        pt = pack[:, kb % 4]
                        nc.scalar.activation(
                            pt[:, off:],
                            ps[:, off:],
                            mybir.ActivationFunctionType.Exp,
                            scale=SCALE,
                        )
                        if j0 >= 0:
                            nc.vector.tensor_tensor(
                                pt[:, j0 * P : (j0 + 1) * P],
                                pt[:, j0 * P : (j0 + 1) * P],
                                utri,
                                mybir.AluOpType.mult,
                            )
                        p_tiles.append(pt)

                def scores(qs):
                    p_tiles = []
                    for g in range(qs + 1):
                        scores_group(qs, g, p_tiles)
                    return p_tiles

                def make_outs(qs, p_tiles):
                    for j in range(QSB // P):
                        qb = qs * (QSB // P) + j
                        po = psB.tile([P, VF], F32, tag="B")
                        for kb in range(qb + 1):
                            nc.tensor.matmul(
                                po,
                                p_tiles[kb][:, j * P : (j + 1) * P],
                                v2[:, kb, :],
                                start=(kb == 0),
                                stop=(kb == qb),
                            )
                        rec = out_pool.tile([P, 1], F32, tag="rec")
                        nc.vector.reciprocal(rec, po[:, H : H + 1])
                        ot = out_pool.tile([P, H], F16, tag="out")
                        nc.vector.tensor_scalar_mul(ot, po[:, :H], rec)
                        nc.sync.dma_start(y_d[qb * P : (qb + 1) * P, :], ot)

                # t-chunks 0/1 as before: project, then attention for the
                # matching superblock (its exps overlap the next chunk's
                # projections on the PE).
                proj_mm(0, 0, qT)
                proj_mm(1, 0, kT)
                proj_v(0)
                sb0_tiles = scores(0)
                proj_mm(0, 1, qT)
                proj_mm(1, 1, kT)
                proj_v(1)
                make_outs(0, sb0_tiles)
                # The endgame is explicitly interleaved so the ACT engine
                # (the serial bottleneck: ~20us of exp for sb1+sb2+sb3) is
                # fed as early as each qT/kT piece exists, while the PE's
                # psA-ring waits are filled with exp-independent work (V
                # projections, deferred out chains).  The tc2 Q-projection
                # is hoisted between sb1's score groups: qT2 then exists
                # ~2us earlier, closing the ACT idle window before sb2.
                sb1_tiles = []
                sb2_tiles = []
                sb3_tiles = []
                scores_group(1, 0, sb1_tiles)
                proj_mm(0, 2, qT)
                scores_group(1, 1, sb1_tiles)
                proj_mm(1, 2, kT)
                scores_group(2, 0, sb2_tiles)
                proj_v(2)
                scores_group(2, 1, sb2_tiles)
                proj_mm(0, 3, qT)
                scores_group(2, 2, sb2_tiles)
                proj_mm(1, 3, kT)
                scores_group(3, 0, sb3_tiles)
                scores_group(3, 1, sb3_tiles)
                proj_v(3)
                scores_group(3, 2, sb3_tiles)
                scores_group(3, 3, sb3_tiles)
                make_outs(1, sb1_tiles)
                make_outs(2, sb2_tiles)
                make_outs(3, sb3_tiles)

    nc.compile()
    return nc


_NC_CACHE = {}


def _get_program():
    if "nc" not in _NC_CACHE:
        _NC_CACHE["nc"] = build_program()
    return _NC_CACHE["nc"]


def _to_bf16_bytes(a):
    """f32 ndarray -> bf16 (round-to-nearest-even) as uint16."""
    u = np.asarray(a, np.float32).view(np.uint32)
    return ((u + 0x7FFF + ((u >> 16) & 1)) >> 16).astype(np.uint16)


def make_in_maps(x, Wq, bq, Wk, bk, Wv, bv):
    x = np.asarray(x, dtype=np.float32)
    blob = np.empty((N_CORES, CB_BYTES + X_BYTES), np.int8)

    # weights: [p, cb, i, h] bf16
    wpack = np.empty((P, NCB, 3, H), np.uint16)
    for i, W in enumerate((Wq, Wk, Wv)):
        w16 = _to_bf16_bytes(np.asarray(W, np.float32))  # [C, H]
        wpack[:, :, i, :] = w16.reshape(NCB, P, H).transpose(1, 0, 2)
    cf = np.zeros((P, 4), np.float32)
    for i, b in enumerate((bq, bk, bv)):
        cf[:, i] = np.asarray(b, dtype=np.float32)
    cf[:, 3] = 1.0

    const_bytes = np.concatenate(
        [wpack.reshape(-1).view(np.int8), cf.reshape(-1).view(np.int8)]
    )
    for b in range(N_CORES):
        blob[b, :CB_BYTES] = const_bytes
        xv = blob[b, CB_BYTES:].view(np.uint16).reshape(C, T)
        xv[:] = _to_bf16_bytes(x[b]).T

    return [{"xb": blob[b]} for b in range(N_CORES)]


def kernel(x, Wq, bq, Wk, bk, Wv, bv):
    nc = _get_program()
    in_maps = make_in_maps(x, Wq, bq, Wk, bk, Wv, bv)
    try:
        res = run_bass_kernel_spmd(nc, in_maps, core_ids=list(range(N_CORES)))
    except Exception:
        # The tunneled device occasionally wedges transiently
        # (NRT_EXEC_UNIT_UNRECOVERABLE); a plain re-run recovers it and
        # results are bit-identical.  A persistent error fails the same
        # way on the retry.
        res = run_bass_kernel_spmd(nc, in_maps, core_ids=list(range(N_CORES)))
    out = np.stack(
        [res.results[b]["y"].astype(np.float32) for b in range(N_CORES)], axis=0
    )
    # the V bias commutes through the softmax-weighted average:
    # attention(V + b) == attention(V) + b -- applied here in exact f32
    out += np.asarray(bv, np.float32)[None, None, :]
    return out


# revision 63
# speedup vs baseline: 1.0260x; 1.0260x over previous
"""Single-head causal attention (B=8, T=2048, C=1024, H=128) on 8 TRN2 cores.

Data-parallel over batch: core b computes attention for x[b].

Device-execution-optimized design:

- The host ships x ALREADY TRANSPOSED as bf16 (x^T [C, T]) so the PE
  consumes it directly: no on-device casts, no per-token dequant
  scales, and no PE transposes of x.  (fp8 DoubleRow was measured on
  hardware at 1.0 cycles/output-column -- 2x flops via the doubled
  contraction, not the 4x the cost model claims -- which makes every
  accuracy-viable fp8 scheme here a wash or a loss vs bf16.)
- All PE matmul operands are bf16 (1 col/cycle at ANY moving width,
  unlike f32r which needs >=256-wide outputs), so the P@V moving width
  is 129 (= H + the row-sum ones column).
- Q^T, K^T, V^T [H, T] = W.T @ x^T as PE matmuls contracting over C
  (8 chunks of 128), bias added during the PSUM->SBUF copyback (single
  DVE tensor_scalar, output bf16).  V^T is PE-transposed back to
  natural V [T, H]; its extra 1.0 column makes the out-matmul carry the
  softmax row-sum for free.
- Scores computed transposed per 512-wide query superblock:
  S^T[k, q] = K^T.T @ Q^T so the softmax reduction lands on the PSUM
  partition dim; P = exp(S^T * scale) on the ACT engine (bf16 out) with
  triangular masking on DVE; one accumulated matmul chain per 128-row
  q-block gives out[q, :H] and the row sum; normalization is a
  per-partition reciprocal multiply on the copyback.  Softmax skips the
  max-subtract: |scores * scale| <= ~0.9 so exp cannot overflow.
- Issue-order pipelining: attention for superblock qs is interleaved
  right after projection t-chunk qs so the big exp batches on the ACT
  engine overlap later PE work.  Out-matmul chains are deferred (sb0 by
  one t-chunk, sb1/sb2 to the end) and the final t-chunk reorders the
  V projection and those deferred chains AFTER the last score matmuls,
  so the PE has work while the ACT engine grinds the last exp batch.
- DMAs are issued in need-order in pieces so the first projection
  matmul gates on ~450KB, not megabytes.
"""

import numpy as np

import concourse.mybir as mybir
import concourse.tile as tile
from concourse import bacc
from concourse.bass_utils import run_bass_kernel_spmd

B, T, C, H = 8, 2048, 1024, 128
P = 128
NCB = C // P  # 8 contraction chunks for the projections
NTB = T // P  # 16 token blocks
TCH = 512  # projection t-chunk width (one PSUM bank)
NTCH = T // TCH  # 4
QSB = 512  # query superblock width for attention
VF = H + 1  # free width of the [V | 1.0] tile
F32 = mybir.dt.float32
BF16 = mybir.dt.bfloat16
F16 = mybir.dt.float16
I8 = mybir.dt.int8
SCALE = float(C) ** -0.5

W_BYTES = P * NCB * 3 * H * 2  # bf16 weights, (p, cb, i, h) layout
CF_BYTES = P * 4 * 4  # f32: bq | bk | bv | 1.0 seed
CB_BYTES = W_BYTES + CF_BYTES
X_BYTES = C * T * 2  # bf16 x^T

N_CORES = 8


def build_program(reps=1):
    nc = bacc.Bacc(
        "TRN2",
        target_bir_lowering=False,
        debug=False,
        enable_asserts=False,
        num_devices=N_CORES,
    )

    xb_d = nc.dram_tensor(
        "xb", (CB_BYTES + X_BYTES,), I8, kind="ExternalInput"
    ).ap()
    # weights: [p, cb, 3, H] bf16; slice (cb, i) = W_i[cb*128 + p, :]
    w_d = xb_d[0:W_BYTES].bitcast(BF16).rearrange(
        "(p cb i h) -> p cb i h", p=P, cb=NCB, i=3
    )
    cf_d = (
        xb_d[W_BYTES:CB_BYTES].bitcast(F32).rearrange("(p c) -> p c", c=4)
    )
    # x^T [C, T] -> [p, cb, t]
    x_d = xb_d[CB_BYTES:].bitcast(BF16).rearrange(
        "(cb p t) -> p cb t", p=P, t=T
    )
    y_d = nc.dram_tensor("y", (T, H), F16, kind="ExternalOutput").ap()

    with tile.TileContext(nc) as tc:
        with (
            tc.tile_pool(name="consts", bufs=1) as consts,
            tc.tile_pool(name="big", bufs=1) as big_pool,
            # P tiles are packed 4 per pool buffer: the tile framework's
            # prologue/epilogue semaphore ladders cost ~115ns per pool
            # buffer on every engine, so fewer, bigger buffers directly
            # shrink the fixed overhead.
            tc.tile_pool(name="ptile", bufs=9) as p_pool,
            tc.tile_pool(name="outs", bufs=4) as out_pool,
            tc.tile_pool(name="psA", bufs=5, space="PSUM") as psA,
            tc.tile_pool(name="psB", bufs=3, space="PSUM") as psB,
        ):
            # DMAs in need-order.  vector finishes its fixed preamble
            # earliest, so it issues the two most critical loads (biases,
            # first weight piece); sync takes the rest of the weights and
            # gpsimd streams x^T (t-chunk 0 in cb-pair pieces, then
            # t-chunks 1-3 whole).
            # PE warm-up: the Tensor engine only reaches its top p-state
            # after ~3us of continuous work, and the first projections
            # otherwise run ~1.5x slow while also waiting on the x DMAs.
            # Burn the DMA wait ramping the clock with throwaway matmuls
            # over a zeroed scratch tile (gated on nothing but a tiny
            # memset that dispatches at ~6us, well before any DMA lands).
            wup = consts.tile([P, TCH], BF16, tag="wup")
            nc.vector.memset(wup, 0.0)
            for _ in range(12):
                psw = psA.tile([P, TCH], F32, tag="A")
                nc.tensor.matmul(
                    psw, wup[:, 0:P], wup, start=True, stop=True
                )

            # All bulk input goes through gpsimd's queue group (the only
            # one that sustains ~270GB/s; sync/scalar groups crawl at
            # ~40GB/s), interleaved in exact consumption order.
            cft = consts.tile([P, 4], F32, tag="cft")
            nc.scalar.dma_start(cft, cf_d)
            wt = consts.tile([P, NCB, 3, H], BF16, tag="wt")
            xt = big_pool.tile([P, NCB, T], BF16, tag="xt")
            wsplit = ((0, 2), (2, 5), (5, 8))
            for cp in range(4):
                if cp < len(wsplit):
                    a, b = wsplit[cp]
                    nc.gpsimd.dma_start(wt[:, a:b], w_d[:, a:b])
                nc.gpsimd.dma_start(
                    xt[:, 2 * cp : 2 * cp + 2, 0:TCH],
                    x_d[:, 2 * cp : 2 * cp + 2, 0:TCH],
                )

            # t-chunks 1 and 2 in cb-pair pieces, interleaved 2:1 --
            # t-chunk 1's consumers have ring-pacing slack, while t-chunk
            # 2 gates qT2 and with it the ACT engine's sb2 exp batch.
            def xpiece(tch, cp):
                nc.gpsimd.dma_start(
                    xt[:, 2 * cp : 2 * cp + 2, tch * TCH : (tch + 1) * TCH],
                    x_d[:, 2 * cp : 2 * cp + 2, tch * TCH : (tch + 1) * TCH],
                )

            for tch, cp in (
                (1, 0), (1, 1), (2, 0), (1, 2), (1, 3), (2, 1), (2, 2), (2, 3),
            ):
                xpiece(tch, cp)
            # mask generated on device from the 1.0 seed column (not
            # needed before ~15us, so issued behind the urgent DMAs)
            utri = consts.tile([P, P], BF16, tag="utri")
            nc.gpsimd.affine_select(
                utri,
                cft[:, 3:4].to_broadcast((P, P)),
                [[1, P]],
                mybir.AluOpType.is_ge,
                0.0,
                base=0,
                channel_multiplier=-1,
            )
            for cp in range(4):  # t-chunk 3 in pieces: the hoisted Q3
                xpiece(3, cp)    # projection pipelines through arrivals

            for _ in range(reps):
                qT = big_pool.tile([P, T], BF16, tag="qT")
                kT = big_pool.tile([P, T], BF16, tag="kT")
                v2 = big_pool.tile([P, NTB, VF], BF16, tag="v2")
                nc.vector.memset(v2[:, :, H : H + 1], 1.0)

                def proj_mm(wi, tch, dst, split=False):
                    tsl = slice(tch * TCH, (tch + 1) * TCH)
                    ps = psA.tile([P, TCH], F32, tag="A")
                    for cb in range(NCB):
                        nc.tensor.matmul(
                            ps,
                            wt[:, cb, wi],
                            xt[:, cb, tsl],
                            start=(cb == 0),
                            stop=(cb == NCB - 1),
                        )
                    nc.vector.tensor_scalar_add(
                        dst[:, tsl], ps, cft[:, wi : wi + 1]
                    )

                def proj_v(tch):
                    # V computed NATURAL directly (lhsT = x block, rhs =
                    # Wv): same matmul columns as the V^T projection but
                    # no PE transposes and no wide copybacks.  The V bias
                    # is NOT applied on device: attention(V + b) ==
                    # attention(V) + b, so the host adds it to the final
                    # output in exact f32.
                    for tb in range(tch * (TCH // P), (tch + 1) * (TCH // P)):
                        po = psB.tile([P, VF], F32, tag="B")
                        for cb in range(NCB):
                            nc.tensor.matmul(
                                po[:, :H],
                                xt[:, cb, tb * P : (tb + 1) * P],
                                wt[:, cb, 2],
                                start=(cb == 0),
                                stop=(cb == NCB - 1),
                            )
                        nc.vector.tensor_copy(v2[:, tb, :H], po[:, :H])

                def scores_group(qs, g, p_tiles):
                    # emit score matmuls + exps for kb in [4g, 4g+4) of
                    # superblock qs; groups of 4 share one P-tile pack
                    pack = p_pool.tile([P, 4, QSB], BF16, tag="P")
                    for kb in range(4 * g, 4 * g + 4):
                        j0 = kb - qs * (QSB // P)  # first valid 128-col block
                        off = max(j0, 0) * P
                        ps = psA.tile([P, QSB], F32, tag="A")
                        nc.tensor.matmul(
                            ps[:, off:],
                            kT[:, kb * P : (kb + 1) * P],
                            qT[:, qs * QSB + off : (qs + 1) * QSB],
                            start=True,
                            stop=True,
                        )
                        pt = pack[:, kb % 4]
                        nc.scalar.activation(
                            pt[:, off:],
                            ps[:, off:],
                            mybir.ActivationFunctionType.Exp,
                            scale=SCALE,
                        )
                        if j0 >= 0:
                            nc.vector.tensor_tensor(
                                pt[:, j0 * P : (j0 + 1) * P],
                                pt[:, j0 * P : (j0 + 1) * P],
                                utri,
                                mybir.AluOpType.mult,
                            )
                        p_tiles.append(pt)

                def scores(qs):
                    p_tiles = []
                    for g in range(qs + 1):
                        scores_group(qs, g, p_tiles)
                    return p_tiles

                def make_outs(qs, p_tiles):
                    for j in range(QSB // P):
                        qb = qs * (QSB // P) + j
                        po = psB.tile([P, VF], F32, tag="B")
                        for kb in range(qb + 1):
                            nc.tensor.matmul(
                                po,
                                p_tiles[kb][:, j * P : (j + 1) * P],
                                v2[:, kb, :],
                                start=(kb == 0),
                                stop=(kb == qb),
                            )
                        rec = out_pool.tile([P, 1], F32, tag="rec")
                        nc.vector.reciprocal(rec, po[:, H : H + 1])
                        ot = out_pool.tile([P, H], F16, tag="out")
                        nc.vector.tensor_scalar_mul(ot, po[:, :H], rec)
                        nc.sync.dma_start(y_d[qb * P : (qb + 1) * P, :], ot)

                # t-chunks 0/1 as before: project, then attention for the
                # matching superblock (its exps overlap the next chunk's
                # projections on the PE).
                proj_mm(0, 0, qT)
                proj_mm(1, 0, kT)
                proj_v(0)
                sb0_tiles = scores(0)
                proj_mm(0, 1, qT)
                proj_mm(1, 1, kT)
                proj_v(1)
                make_outs(0, sb0_tiles)
                sb1_tiles = scores(1)
                # The endgame is explicitly interleaved so the ACT engine
                # (the serial bottleneck: ~20us of exp for sb2+sb3) is fed
                # as early as each qT/kT piece exists, while the PE's
                # psA-ring waits are filled with exp-independent work (V
                # projections, deferred out chains).
                sb2_tiles = []
                sb3_tiles = []
                proj_mm(0, 2, qT)
                proj_mm(1, 2, kT)
                scores_group(2, 0, sb2_tiles)
                proj_v(2)
                scores_group(2, 1, sb2_tiles)
                proj_mm(0, 3, qT)
                scores_group(2, 2, sb2_tiles)
                proj_mm(1, 3, kT)
                scores_group(3, 0, sb3_tiles)
                scores_group(3, 1, sb3_tiles)
                proj_v(3)
                scores_group(3, 2, sb3_tiles)
                scores_group(3, 3, sb3_tiles)
                make_outs(1, sb1_tiles)
                make_outs(2, sb2_tiles)
                make_outs(3, sb3_tiles)

    nc.compile()
    return nc


_NC_CACHE = {}


def _get_program():
    if "nc" not in _NC_CACHE:
        _NC_CACHE["nc"] = build_program()
    return _NC_CACHE["nc"]


def _to_bf16_bytes(a):
    """f32 ndarray -> bf16 (round-to-nearest-even) as uint16."""
    u = np.asarray(a, np.float32).view(np.uint32)
    return ((u + 0x7FFF + ((u >> 16) & 1)) >> 16).astype(np.uint16)


def make_in_maps(x, Wq, bq, Wk, bk, Wv, bv):
    x = np.asarray(x, dtype=np.float32)
    blob = np.empty((N_CORES, CB_BYTES + X_BYTES), np.int8)

    # weights: [p, cb, i, h] bf16
    wpack = np.empty((P, NCB, 3, H), np.uint16)
    for i, W in enumerate((Wq, Wk, Wv)):
        w16 = _to_bf16_bytes(np.asarray(W, np.float32))  # [C, H]
        wpack[:, :, i, :] = w16.reshape(NCB, P, H).transpose(1, 0, 2)
    cf = np.zeros((P, 4), np.float32)
    for i, b in enumerate((bq, bk, bv)):
        cf[:, i] = np.asarray(b, dtype=np.float32)
    cf[:, 3] = 1.0

    const_bytes = np.concatenate(
        [wpack.reshape(-1).view(np.int8), cf.reshape(-1).view(np.int8)]
    )
    for b in range(N_CORES):
        blob[b, :CB_BYTES] = const_bytes
        xv = blob[b, CB_BYTES:].view(np.uint16).reshape(C, T)
        xv[:] = _to_bf16_bytes(x[b]).T

    return [{"xb": blob[b]} for b in range(N_CORES)]


def kernel(x, Wq, bq, Wk, bk, Wv, bv):
    nc = _get_program()
    in_maps = make_in_maps(x, Wq, bq, Wk, bk, Wv, bv)
    try:
        res = run_bass_kernel_spmd(nc, in_maps, core_ids=list(range(N_CORES)))
    except Exception:
        # The tunneled device occasionally wedges transiently
        # (NRT_EXEC_UNIT_UNRECOVERABLE); a plain re-run recovers it and
        # results are bit-identical.  A persistent error fails the same
        # way on the retry.
        res = run_bass_kernel_spmd(nc, in_maps, core_ids=list(range(N_CORES)))
    out = np.stack(
        [res.results[b]["y"].astype(np.float32) for b in range(N_CORES)], axis=0
    )
    # the V bias commutes through the softmax-weighted average:
    # attention(V + b) == attention(V) + b -- applied here in exact f32
    out += np.asarray(bv, np.float32)[None, None, :]
    return out
